# revision 5
# baseline (speedup 1.0000x reference)
"""Sparse 3D deconvolution (transpose conv) on 8 Trainium2 NeuronCores.

Algorithm (output/data parallel, per the sharding hint):
  - Host builds the voxel grid lookup and, per core, the compact hit list
    (out_row, kernel_offset k, input_row) for its 12.5k-output-row shard.
  - Hits are sorted by (round, k) where round = ordinal of the contribution
    within its output row.  Each round therefore touches each output row at
    most once, so per-round DMA scatter-adds are free of same-row races
    (rounds themselves serialize through Tile's DRAM WAW tracking).
  - Device: dma_gather feature rows (int16-indexed compact table) ->
    PE-transpose to put C_IN on partitions -> per-(round,k)-run matmuls with
    W[26-k] stationary producing out^T in PSUM -> PE-transpose back ->
    per-round dma_scatter_add into the output shard (zero-initialized).
  - The run-length grid (max across cores, padded) is shared so one SPMD
    program serves all 8 cores.
"""

import numpy as np

G = 128
KK = 27
CIN = 128
COUT = 128
NCORES = 8
CHUNK = 512          # tokens per gather / psum tile
DUMMY = 128          # scratch output rows for pad-token scatters


def _kernel_offsets():
    r = np.arange(3) - 1
    dx, dy, dz = np.meshgrid(r, r, r, indexing="ij")
    return np.stack([dx.ravel(), dy.ravel(), dz.ravel()], axis=1).astype(np.int64)


def _host_prep(features, W, inp_positions, out_positions):
    n_in = features.shape[0]
    n_out = out_positions.shape[0]
    shard = n_out // NCORES
    assert n_out % NCORES == 0

    ip = inp_positions.astype(np.int64)
    op = out_positions.astype(np.int64)

    lin_in = (ip[:, 0] * G + ip[:, 1]) * G + ip[:, 2]
    grid = np.full(G * G * G, -1, np.int64)
    np.maximum.at(grid, lin_in, np.arange(n_in, dtype=np.int64))

    offs = _kernel_offsets()
    nb = op[:, None, :] + offs[None, :, :]          # [n_out, 27, 3]
    inb = np.all((nb >= 0) & (nb < G), axis=2)
    lin = (nb[..., 0] * G + nb[..., 1]) * G + nb[..., 2]
    idx = grid[np.clip(lin, 0, G * G * G - 1)]
    ok = inb & (idx >= 0)

    o_all, k_all = np.nonzero(ok)                   # sorted by o, then k
    in_all = idx[o_all, k_all]

    first = np.r_[True, o_all[1:] != o_all[:-1]]
    run_starts = np.nonzero(first)[0]
    seg_len = np.diff(np.r_[run_starts, len(o_all)])
    r_all = np.arange(len(o_all)) - np.repeat(run_starts, seg_len)

    core_all = o_all // shard
    o_loc = o_all % shard
    R_max = int(r_all.max()) + 1 if len(r_all) else 1

    counts = np.zeros((NCORES, R_max, KK), np.int64)
    np.add.at(counts, (core_all, r_all, k_all), 1)
    L = counts.max(axis=0)                          # [R_max, KK] common run lengths
    round_tok = L.sum(axis=1)
    R_r = ((round_tok + 127) // 128) * 128
    pad_runs = R_r - round_tok

    runs = []
    round_starts = []
    pos = 0
    grp_start = {}
    for r in range(R_max):
        round_starts.append(pos)
        for k in range(KK):
            if L[r, k] > 0:
                grp_start[(r, k)] = pos
                runs.append((int(k), int(pos), int(L[r, k])))
                pos += int(L[r, k])
        if pad_runs[r] > 0:
            runs.append((26, int(pos), int(pad_runs[r])))
            pos += int(pad_runs[r])
    H_pad = pos
    if H_pad % CHUNK:
        extra = CHUNK - H_pad % CHUNK
        runs.append((26, H_pad, int(extra)))
        H_pad += extra
    runs_split = []
    for k, s, ln in runs:
        while ln > 0:
            take = min(ln, CHUNK - (s % CHUNK))
            runs_split.append((k, s, take))
            s += take
            ln -= take
    rounds = [(int(round_starts[r]), int(R_r[r])) for r in range(R_max)]

    order = np.lexsort((o_loc, k_all, r_all, core_all))
    oc, rc, kc, olc, inc = (core_all[order], r_all[order], k_all[order],
                            o_loc[order], in_all[order])
    per_core = []
    for c in range(NCORES):
        m = oc == c
        rcc, kcc, olcc, incc = rc[m], kc[m], olc[m], inc[m]
        uniq, inv = np.unique(incc, return_inverse=True)
        gidx = np.zeros(H_pad, np.int64)            # 0 -> zero row
        sidx = shard + (np.arange(H_pad) % DUMMY)   # default: pad -> dummy rows
        if len(rcc):
            gb = np.r_[True, (rcc[1:] != rcc[:-1]) | (kcc[1:] != kcc[:-1])]
            gs = np.nonzero(gb)[0]
            glen = np.diff(np.r_[gs, len(rcc)])
            within = np.arange(len(rcc)) - np.repeat(gs, glen)
            starts = np.array([grp_start[(int(r_), int(k_))]
                               for r_, k_ in zip(rcc[gs], kcc[gs])])
            tok = np.repeat(starts, glen) + within
            gidx[tok] = inv + 1
            sidx[tok] = olcc
        per_core.append(dict(uniq=uniq, gidx=gidx, sidx=sidx))

    U_pad = ((max(len(p["uniq"]) for p in per_core) + 1 + 127) // 128) * 128
    assert U_pad < 32768 and shard + DUMMY < 32768, "int16 index overflow"
    for p in per_core:
        Fc = np.zeros((U_pad, CIN), np.float32)
        Fc[1:1 + len(p["uniq"])] = features[p["uniq"]]
        p["Fc"] = Fc

    # W in SBUF layout: [C_IN partitions, 27*C_OUT], column block k = W[26-k]
    Wm = W[::-1].astype(np.float32)                 # [27, C_IN, C_OUT]
    W_sb = np.ascontiguousarray(Wm.transpose(1, 0, 2).reshape(CIN, KK * COUT))
    return per_core, runs_split, rounds, H_pad, U_pad, W_sb, shard


def _wrap16(a):
    """[H] int -> [128, H//16] int16, token i at [i%16, i//16], replicated x8."""
    s = len(a) // 16
    w = a.reshape(s, 16).T.astype(np.int16)
    return np.ascontiguousarray(np.tile(w, (8, 1)))


def _build_program(runs_split, rounds, H_pad, U_pad, shard):
    from concourse import bacc, tile, mybir
    from concourse.masks import make_identity

    dt = mybir.dt
    nc = bacc.Bacc("TRN2", target_bir_lowering=False, debug=False,
                   num_devices=NCORES, dynamic_dma_scratch_size=32768)
    fc_d = nc.dram_tensor("fc", [U_pad, CIN], dt.float32, kind="ExternalInput")
    w_d = nc.dram_tensor("wsb", [CIN, KK * COUT], dt.float32, kind="ExternalInput")
    gi_d = nc.dram_tensor("gidx", [128, H_pad // 16], dt.int16, kind="ExternalInput")
    si_d = nc.dram_tensor("sidx", [128, H_pad // 16], dt.int16, kind="ExternalInput")
    out_d = nc.dram_tensor("out", [shard + DUMMY, COUT], dt.float32,
                           kind="ExternalOutput")

    nchunks = H_pad // CHUNK
    runs_by_chunk = [[] for _ in range(nchunks)]
    for k, s, ln in runs_split:
        runs_by_chunk[s // CHUNK].append((k, s % CHUNK, ln))

    with tile.TileContext(nc) as tc:
        with (
            tc.tile_pool(name="const", bufs=1) as constp,
            tc.tile_pool(name="gbuf", bufs=3) as gpool,
            tc.tile_pool(name="ft", bufs=3) as ftpool,
            tc.tile_pool(name="xb", bufs=4) as xpool,
            tc.tile_pool(name="sglob", bufs=1) as spool,
            tc.tile_pool(name="pt", bufs=2, space="PSUM") as ptpool,
            tc.tile_pool(name="po", bufs=2, space="PSUM") as popool,
            tc.tile_pool(name="py", bufs=2, space="PSUM") as pypool,
        ):
            ident = constp.tile([128, 128], dt.float32)
            make_identity(nc, ident[:])
            wsb = constp.tile([128, KK * COUT], dt.float32)
            nc.sync.dma_start(out=wsb[:], in_=w_d.ap())
            gi_sb = constp.tile([128, H_pad // 16], dt.int16)
            nc.sync.dma_start(out=gi_sb[:], in_=gi_d.ap())
            si_sb = constp.tile([128, H_pad // 16], dt.int16)
            nc.sync.dma_start(out=si_sb[:], in_=si_d.ap())

            s_glob = spool.tile([128, (H_pad // 128) * COUT], dt.float32)

            for ci in range(nchunks):
                gt = gpool.tile([128, (CHUNK // 128) * CIN], dt.float32)
                nc.gpsimd.dma_gather(
                    out_ap=gt[:].rearrange("p (t e) -> p t e", e=CIN),
                    in_ap=fc_d.ap(),
                    idxs_ap=gi_sb[:, ci * (CHUNK // 16):(ci + 1) * (CHUNK // 16)],
                    num_idxs=CHUNK,
                    num_idxs_reg=CHUNK,
                    elem_size=CIN,
                )
                ft = ftpool.tile([128, CHUNK], dt.float32)
                for s in range(CHUNK // 128):
                    pt = ptpool.tile([128, 128], dt.float32)
                    nc.tensor.transpose(pt[:], gt[:, s * CIN:(s + 1) * CIN], ident[:])
                    nc.vector.tensor_copy(out=ft[:, s * 128:(s + 1) * 128], in_=pt[:])
                po = popool.tile([128, CHUNK], dt.float32)
                for k, s0, ln in runs_by_chunk[ci]:
                    nc.tensor.matmul(
                        out=po[:, s0:s0 + ln],
                        lhsT=wsb[:, k * COUT:(k + 1) * COUT],
                        rhs=ft[:, s0:s0 + ln],
                        start=True, stop=True,
                    )
                for s in range(CHUNK // 128):
                    x = xpool.tile([128, 128], dt.float32)
                    nc.scalar.copy(out=x[:], in_=po[:, s * 128:(s + 1) * 128])
                    py = pypool.tile([128, 128], dt.float32)
                    nc.tensor.transpose(py[:], x[:], ident[:])
                    blk = ci * (CHUNK // 128) + s
                    nc.vector.tensor_copy(
                        out=s_glob[:, blk * COUT:(blk + 1) * COUT], in_=py[:])

            for rs, rl in rounds:
                nc.gpsimd.dma_scatter_add(
                    out_ap=out_d.ap(),
                    in_ap=s_glob[:, (rs // 128) * COUT:((rs + rl) // 128) * COUT]
                        .rearrange("p (t e) -> p t e", e=COUT),
                    idxs_ap=si_sb[:, rs // 16:(rs + rl) // 16],
                    num_idxs=rl,
                    num_idxs_reg=rl,
                    elem_size=COUT,
                )
    nc.compile()
    return nc


LAST = {}


def kernel(features, W, inp_positions, out_positions):
    features = np.asarray(features, dtype=np.float32)
    W = np.asarray(W, dtype=np.float32)
    inp_positions = np.asarray(inp_positions)
    out_positions = np.asarray(out_positions)
    n_out = out_positions.shape[0]

    per_core, runs_split, rounds, H_pad, U_pad, W_sb, shard = _host_prep(
        features, W, inp_positions, out_positions)

    nc = _build_program(runs_split, rounds, H_pad, U_pad, shard)

    in_maps = []
    for c in range(NCORES):
        p = per_core[c]
        in_maps.append({
            "fc": p["Fc"],
            "wsb": W_sb,
            "gidx": _wrap16(p["gidx"]),
            "sidx": _wrap16(p["sidx"]),
        })

    from concourse import bass_utils
    import os
    trace = bool(int(os.environ.get("KERNEL_TRACE", "0")))
    res = bass_utils.run_bass_kernel_spmd(
        nc, in_maps, core_ids=list(range(NCORES)), trace=trace)
    LAST["exec_time_ns"] = res.exec_time_ns
    LAST["results"] = res

    out = np.concatenate([res.results[c]["out"][:shard] for c in range(NCORES)],
                         axis=0)
    assert out.shape == (n_out, COUT)
    return out.astype(np.float32)
